# revision 20
# baseline (speedup 1.0000x reference)
"""DFL loss (nn_DFLLoss) Trainium2 Bass kernel — 8-core data parallel.

reference computes, per (batch, pixel, coord j in 0..3):
    rl[b, hw, j, k] = reg_logits[b, j*8+k, hw]          (k in 0..7 bins)
    t = clip(targets, 0, 6.9999); l = floor(t); u = l+1
    per = w_l * (lse - rl[l]) + w_u * (lse - rl[u]),  lse = logsumexp_k rl
    loss = sum(per * pos_mask) / (max(sum(pos_mask), 1) * 4)

Key identity used here (removes the gather):
    w_l*rl[l] + w_u*rl[u] = sum_k relu(1 - |t - k|) * rl[k]
so masked_total = sum(mask*lse) - sum_k relu(1-|t-k|)*rl[k]*mask. The
hat-product+reduce runs as ONE fused custom DVE op per (batch, coord)
with the bin index k supplied by PageIdx over the 8 channel pages.
The mask is folded into t'' = clip(t) + 100*mask and the op evaluates
relu(1 - |t'' - (100 + k)|): positive pixels give |t - k|, masked-out
pixels give |t - 100 - k| >= 93 so every hat weight is exactly 0.

Per-core layout (4 batches, processed as 4 pipeline phases): partition
p = pixel-block (HW = 25600 = 128 blocks x 200 px). Channels live in the
free dimension, so Sum_k exp(rl) is a 3-level pairwise tree of bf16
tensor_tensor adds (2x mode), exp/ln run full-width on ScalarE, and the
small masked accumulations run on GpSimd (which never contends with the
1-port DVE op mix used here).
"""

import threading
from operator import add as _operator_add

import numpy as np

BINS = 8
B, C, H, W = 32, 32, 160, 160
HW = H * W  # 25600
NCORES = 8
BPC = B // NCORES  # 4 batches per core
PX = HW // 128  # 200 pixels per partition per batch
NJ = 4

_lock = threading.Lock()
_cache: dict = {}


def _register_hat_op():
    """Register the fused hat*logit+reduce custom DVE op (idempotent)."""
    import concourse.dve_ops as dve_ops
    from concourse.dve_spec import (
        C0,
        C1,
        PageIdx,
        Spec,
        Src0,
        Src1,
        Zero,
        One,
        lower,
        maxx,
        relu,
    )
    from concourse.dve_uop import DveOpSpec

    name = "HAT_MUL_ACC_DFL"
    if name in dve_ops._SUB_OPCODE_FOR_NAME:
        for op in dve_ops.OPS:
            if op.name == name:
                return op

    _pg = PageIdx(C0, C1)  # idx = s0 + s1*page  (page = bin k)
    _d = Src0 - _pg

    def _ref(in0, in1, s0, s1, imm2):
        P, S, N = in0.shape
        idx = (s0 + s1 * np.arange(S)).reshape(1, S, 1)
        hat = np.maximum(1.0 - np.abs(in0.astype(np.float32) - idx), 0.0)
        body = (hat * in1).astype(np.float32)
        return body, body.reshape(P, -1).sum(-1, keepdims=True)

    spec = Spec(
        body=relu(One - maxx(_d, Zero - _d)) * Src1,
        accum=_operator_add,
        accum_init=Zero,
        reference=_ref,
    )
    shas = {}
    for ver in ("v3", "v4"):
        uops = lower(spec, ver=ver)
        shas[ver] = DveOpSpec(name=name, opcode=1, uops=uops, rd1_en=True).sha(ver)
    op = dve_ops.DveOp(name, spec, subdim=True, uops_sha=shas)
    row = dve_ops._CUSTOM_DVE_ROW_BASE + len(dve_ops.OPS)
    assert row < 0x20, "custom DVE opcode rows exhausted"
    dve_ops.OPS.append(op)
    dve_ops.CUSTOM_DVE_SPECS[name] = op.spec
    dve_ops._SUB_OPCODE_FOR_NAME[name] = row
    return op


def _patch_act_tables():
    """Force Exp and Ln to resolve to the one table set containing both.

    The act-table-load pass assigns each activation the first set containing
    its function; Exp->exp_and_others and Ln->natural_log would otherwise
    alternate table loads (~1.3us each) on every exp->ln transition. Removing
    the two functions from every other set (list order and ids preserved)
    makes natural_log_exp_and_others serve both: one load for the kernel.
    """
    import concourse.bacc as bacc
    import concourse.hw_specs as hw_specs
    import concourse.mybir as mybir

    if getattr(_patch_act_tables, "_done", False):
        return
    orig = hw_specs.get_activation_tables
    Exp = mybir.ActivationFunctionType.Exp
    Ln = mybir.ActivationFunctionType.Ln

    def patched(module_arch):
        t = orig(module_arch)
        both = t.get("natural_log_exp_and_others")
        if both is not None and Exp in both and Ln in both:
            for name, fns in t.items():
                if name != "natural_log_exp_and_others":
                    fns.discard(Exp)
                    fns.discard(Ln)
        return t

    hw_specs.get_activation_tables = patched
    bacc.get_activation_tables = patched
    _patch_act_tables._done = True


def _build_nc():
    import concourse.bacc as bacc
    import concourse.mybir as mybir
    from concourse.tile import TileContext
    from concourse.dve_ops import TENSOR_TENSOR_REDUCE as ttr_op

    _patch_act_tables()
    hat_op = _register_hat_op()
    f32 = mybir.dt.float32
    bf16 = mybir.dt.bfloat16
    u8 = mybir.dt.uint8

    nc = bacc.Bacc("TRN2", target_bir_lowering=False, debug=False)
    x = nc.dram_tensor("x", [BPC, C, HW], f32, kind="ExternalInput")
    tg = nc.dram_tensor("tg", [BPC, HW, NJ], f32, kind="ExternalInput")
    mk = nc.dram_tensor("mk", [BPC, HW], u8, kind="ExternalInput")
    # acc columns: [0:16] interp (b*4+j), [16:32] lse (b*4+j), [32:36] npos100
    acc_out = nc.dram_tensor("acc", [128, 36], f32, kind="ExternalOutput")

    # DRAM views (per batch): partition p = pixel-block of 200 px
    x_v = x.rearrange("b c (blk px) -> b blk c px", px=PX)  # [4,128,32,200]
    tg_v = tg.rearrange("b (blk pj) j -> b blk (pj j)", blk=128)  # [4,128,800]
    mk_v = mk.rearrange("b (blk px) -> b blk px", px=PX)  # [4,128,200]

    Exp = mybir.ActivationFunctionType.Exp
    Ln = mybir.ActivationFunctionType.Ln
    Alu = mybir.AluOpType

    with TileContext(nc) as tc:
        with (
            tc.tile_pool(name="pL", bufs=5) as pL,
            tc.tile_pool(name="pE", bufs=4) as pE,
            tc.tile_pool(name="pS", bufs=5) as pS,
            tc.tile_pool(name="pHat", bufs=4) as pHat,
            tc.tile_pool(name="pT", bufs=3) as pT,
            tc.tile_pool(name="pOnce", bufs=1) as pOnce,
        ):
            accs = pOnce.tile([128, 36], f32)

            for b in range(BPC):
                # --- per-batch setup: targets, mask ---
                # NOTE: several TPB structs (TensorScalarPtr/STT) encode only
                # ONE sync-wait command, so each op below is arranged to need
                # at most one cross-engine semaphore wait.
                t_raw = pT.tile([128, PX * NJ], f32, tag="t_raw")  # (px, j)
                m_raw = pT.tile([128, PX], u8, tag="m_raw")
                mf100 = pT.tile([128, PX], f32, tag="mf100")
                tclp = pT.tile([128, NJ, PX], f32, tag="tclp")  # clipped t, j-major
                t2 = pT.tile([128, NJ, PX], f32, tag="t2")  # t'' j-major

                nc.sync.dma_start(t_raw[:, :], tg_v[b])
                nc.sync.dma_start(m_raw[:, :], mk_v[b])

                # prep engine: DVE for batch 0 (shortest pipeline ramp),
                # GpSimd afterwards (keeps DVE free for the custom ops)
                prep = nc.vector if b == 0 else nc.gpsimd
                # mf100 = 100 * mask (waits only on the mask DMA)
                prep.tensor_scalar(
                    out=mf100[:, :],
                    in0=m_raw[:, :],
                    scalar1=100.0,
                    scalar2=None,
                    op0=Alu.mult,
                )
                # tc = min(t, 6.9999), j-major (waits only on t DMA)
                t_raw_v = t_raw[:, :].rearrange("p (px j) -> p j px", j=NJ)
                prep.tensor_scalar(
                    out=tclp[:, :, :],
                    in0=t_raw_v,
                    scalar1=float(BINS - 1) - 1e-4,
                    scalar2=None,
                    op0=Alu.min,
                )
                # t'' = tc + 100*mask (same-engine deps only)
                prep.tensor_tensor(
                    out=t2[:, :, :],
                    in0=tclp[:, :, :],
                    in1=mf100[:, :].unsqueeze(1).broadcast_to((128, NJ, PX)),
                    op=Alu.add,
                )
                # npos accum (DVE; same-engine dep on mf100 only)
                np_scr = pT.tile([128, PX], f32, tag="np_scr")
                nc.vector.tensor_scalar(
                    out=np_scr[:, :],
                    in0=mf100[:, :],
                    scalar1=0.01,
                    scalar2=0.0,
                    op0=Alu.mult,
                    op1=Alu.add,  # reduce op for accum_out
                    accum_out=accs[:, 32 + b : 33 + b],
                )

                for j in range(NJ):
                    u = b * NJ + j
                    L = pL.tile([128, BINS, PX], f32, tag="L")
                    nc.sync.dma_start(L[:, :, :], x_v[b, :, 8 * j : 8 * j + 8, :])

                    # interp: acc[:, u] = sum_k relu(1-|t-k|) * L_k
                    hat_scr = pHat.tile([128, BINS, PX], bf16, tag="hat")
                    nc.vector._custom_dve(
                        hat_op,
                        out=hat_scr[:, :, :],
                        in0=t2[:, j, :].unsqueeze(1).broadcast_to((128, BINS, PX)),
                        in1=L[:, :, :],
                        s0=100.0,
                        s1=1.0,
                        accum_out=accs[:, u : u + 1],
                    )

                    # lse: exp -> pairwise tree -> ln -> masked accumulate
                    E = pE.tile([128, BINS, PX], bf16, tag="E")
                    nc.scalar.activation(E[:, :, :], L[:, :, :], Exp)
                    s16 = pS.tile([128, 4, PX], bf16, tag="s16")
                    s16_eng = nc.gpsimd if (u % 2 == 0) else nc.vector
                    s16_eng.tensor_tensor(
                        out=s16[:, :, :],
                        in0=E[:, 0::2, :],
                        in1=E[:, 1::2, :],
                        op=Alu.add,
                    )
                    s8 = pS.tile([128, 2, PX], bf16, tag="s8")
                    s8_eng = nc.gpsimd if (u % 4 == 1) else nc.vector
                    s8_eng.tensor_tensor(
                        out=s8[:, :, :],
                        in0=s16[:, 0::2, :],
                        in1=s16[:, 1::2, :],
                        op=Alu.add,
                    )
                    s4 = pS.tile([128, PX], f32, tag="s4")
                    nc.vector.tensor_tensor(
                        out=s4[:, :],
                        in0=s8[:, 0, :],
                        in1=s8[:, 1, :],
                        op=Alu.add,
                    )
                    lse = pS.tile([128, PX], f32, tag="lse")
                    nc.scalar.activation(lse[:, :], s4[:, :], Ln)
                    # acc[:, 16+u] = sum(lse * mf100 * 0.01) = sum(lse * mask)
                    lse_scr = pS.tile([128, PX], f32, tag="lse_scr")
                    nc.vector._custom_dve(
                        ttr_op,
                        out=lse_scr[:, :],
                        in0=lse[:, :],
                        in1=mf100[:, :],
                        s0=0.0,
                        s1=0.01,
                        accum_out=accs[:, 16 + u : 17 + u],
                    )

            nc.sync.dma_start(acc_out[:, :], accs[:, :])

    nc.finalize()
    return nc


def _get_nc():
    with _lock:
        if "nc" not in _cache:
            _cache["nc"] = _build_nc()
        return _cache["nc"]


def kernel(reg_logits: np.ndarray, targets: np.ndarray, pos_mask: np.ndarray) -> np.ndarray:
    from concourse.bass_utils import run_bass_kernel_spmd

    nc = _get_nc()

    reg_logits = np.ascontiguousarray(reg_logits, dtype=np.float32).reshape(B, C, HW)
    targets = np.ascontiguousarray(targets, dtype=np.float32)
    mask_u8 = np.ascontiguousarray(pos_mask).astype(np.uint8)

    in_maps = []
    for c in range(NCORES):
        b0 = c * BPC
        in_maps.append(
            {
                "x": reg_logits[b0 : b0 + BPC],
                "tg": targets[b0 : b0 + BPC],
                "mk": mask_u8[b0 : b0 + BPC],
            }
        )

    res = run_bass_kernel_spmd(nc, in_maps, core_ids=list(range(NCORES)))

    tot_interp = 0.0
    tot_lse = 0.0
    npos100 = 0.0
    for r in res.results:
        a = r["acc"].astype(np.float64)
        tot_interp += a[:, :16].sum()
        tot_lse += a[:, 16:32].sum()
        npos100 += a[:, 32:36].sum()

    npos = npos100  # npos accum already scaled to counts
    total = tot_lse - tot_interp
    loss = total / (max(npos, 1.0) * 4.0) if npos > 0 else 0.0
    return np.float32(loss)


if __name__ == "__main__":
    rng = np.random.default_rng(0)
    rl = rng.standard_normal((B, C, H, W), dtype=np.float32)
    tg = (rng.random((B, HW, NJ), dtype=np.float32) * (BINS - 1)).astype(np.float32)
    pm = rng.integers(0, 2, size=(B, HW)).astype(bool)
    print(kernel(reg_logits=rl, targets=tg, pos_mask=pm))


# revision 21
# speedup vs baseline: 1.0041x; 1.0041x over previous
"""DFL loss (nn_DFLLoss) Trainium2 Bass kernel — 8-core data parallel.

reference computes, per (batch, pixel, coord j in 0..3):
    rl[b, hw, j, k] = reg_logits[b, j*8+k, hw]          (k in 0..7 bins)
    t = clip(targets, 0, 6.9999); l = floor(t); u = l+1
    per = w_l * (lse - rl[l]) + w_u * (lse - rl[u]),  lse = logsumexp_k rl
    loss = sum(per * pos_mask) / (max(sum(pos_mask), 1) * 4)

Key identity used here (removes the gather):
    w_l*rl[l] + w_u*rl[u] = sum_k relu(1 - |t - k|) * rl[k]
so masked_total = sum(mask*lse) - sum_k relu(1-|t-k|)*rl[k]*mask. The
hat-product+reduce runs as ONE fused custom DVE op per (batch, coord)
with the bin index k supplied by PageIdx over the 8 channel pages.
The mask is folded into t'' = clip(t) + 100*mask and the op evaluates
relu(1 - |t'' - (100 + k)|): positive pixels give |t - k|, masked-out
pixels give |t - 100 - k| >= 93 so every hat weight is exactly 0.

Per-core layout (4 batches, processed as 4 pipeline phases): partition
p = pixel-block (HW = 25600 = 128 blocks x 200 px). Channels live in the
free dimension, so Sum_k exp(rl) is a 3-level pairwise tree of bf16
tensor_tensor adds (2x mode), exp/ln run full-width on ScalarE, and the
small masked accumulations run on GpSimd (which never contends with the
1-port DVE op mix used here).
"""

import threading
from operator import add as _operator_add

import numpy as np

BINS = 8
B, C, H, W = 32, 32, 160, 160
HW = H * W  # 25600
NCORES = 8
BPC = B // NCORES  # 4 batches per core
PX = HW // 128  # 200 pixels per partition per batch
NJ = 4

_lock = threading.Lock()
_cache: dict = {}


def _register_hat_op():
    """Register the fused hat*logit+reduce custom DVE op (idempotent)."""
    import concourse.dve_ops as dve_ops
    from concourse.dve_spec import (
        C0,
        C1,
        PageIdx,
        Spec,
        Src0,
        Src1,
        Zero,
        One,
        lower,
        maxx,
        relu,
    )
    from concourse.dve_uop import DveOpSpec

    name = "HAT_MUL_ACC_DFL"
    if name in dve_ops._SUB_OPCODE_FOR_NAME:
        for op in dve_ops.OPS:
            if op.name == name:
                return op

    _pg = PageIdx(C0, C1)  # idx = s0 + s1*page  (page = bin k)
    _d = Src0 - _pg

    def _ref(in0, in1, s0, s1, imm2):
        P, S, N = in0.shape
        idx = (s0 + s1 * np.arange(S)).reshape(1, S, 1)
        hat = np.maximum(1.0 - np.abs(in0.astype(np.float32) - idx), 0.0)
        body = (hat * in1).astype(np.float32)
        return body, body.reshape(P, -1).sum(-1, keepdims=True)

    spec = Spec(
        body=relu(One - maxx(_d, Zero - _d)) * Src1,
        accum=_operator_add,
        accum_init=Zero,
        reference=_ref,
    )
    shas = {}
    for ver in ("v3", "v4"):
        uops = lower(spec, ver=ver)
        shas[ver] = DveOpSpec(name=name, opcode=1, uops=uops, rd1_en=True).sha(ver)
    op = dve_ops.DveOp(name, spec, subdim=True, uops_sha=shas)
    row = dve_ops._CUSTOM_DVE_ROW_BASE + len(dve_ops.OPS)
    assert row < 0x20, "custom DVE opcode rows exhausted"
    dve_ops.OPS.append(op)
    dve_ops.CUSTOM_DVE_SPECS[name] = op.spec
    dve_ops._SUB_OPCODE_FOR_NAME[name] = row
    return op


def _patch_act_tables():
    """Force Exp and Ln to resolve to the one table set containing both.

    The act-table-load pass assigns each activation the first set containing
    its function; Exp->exp_and_others and Ln->natural_log would otherwise
    alternate table loads (~1.3us each) on every exp->ln transition. Removing
    the two functions from every other set (list order and ids preserved)
    makes natural_log_exp_and_others serve both: one load for the kernel.
    """
    import concourse.bacc as bacc
    import concourse.hw_specs as hw_specs
    import concourse.mybir as mybir

    if getattr(_patch_act_tables, "_done", False):
        return
    orig = hw_specs.get_activation_tables
    Exp = mybir.ActivationFunctionType.Exp
    Ln = mybir.ActivationFunctionType.Ln

    def patched(module_arch):
        t = orig(module_arch)
        both = t.get("natural_log_exp_and_others")
        if both is not None and Exp in both and Ln in both:
            for name, fns in t.items():
                if name != "natural_log_exp_and_others":
                    fns.discard(Exp)
                    fns.discard(Ln)
        return t

    hw_specs.get_activation_tables = patched
    bacc.get_activation_tables = patched
    _patch_act_tables._done = True


def _build_nc():
    import concourse.bacc as bacc
    import concourse.mybir as mybir
    from concourse.tile import TileContext
    from concourse.dve_ops import TENSOR_TENSOR_REDUCE as ttr_op

    _patch_act_tables()
    hat_op = _register_hat_op()
    f32 = mybir.dt.float32
    bf16 = mybir.dt.bfloat16
    u8 = mybir.dt.uint8

    nc = bacc.Bacc("TRN2", target_bir_lowering=False, debug=False)
    x = nc.dram_tensor("x", [BPC, C, HW], f32, kind="ExternalInput")
    tg = nc.dram_tensor("tg", [BPC, HW, NJ], f32, kind="ExternalInput")
    mk = nc.dram_tensor("mk", [BPC, HW], u8, kind="ExternalInput")
    # acc columns: [0:16] interp (b*4+j), [16:32] lse (b*4+j), [32:36] npos100
    acc_out = nc.dram_tensor("acc", [128, 36], f32, kind="ExternalOutput")

    # DRAM views (per batch): partition p = pixel-block of 200 px
    x_v = x.rearrange("b c (blk px) -> b blk c px", px=PX)  # [4,128,32,200]
    tg_v = tg.rearrange("b (blk pj) j -> b blk (pj j)", blk=128)  # [4,128,800]
    mk_v = mk.rearrange("b (blk px) -> b blk px", px=PX)  # [4,128,200]

    Exp = mybir.ActivationFunctionType.Exp
    Ln = mybir.ActivationFunctionType.Ln
    Alu = mybir.AluOpType

    with TileContext(nc) as tc:
        with (
            tc.tile_pool(name="pL", bufs=5) as pL,
            tc.tile_pool(name="pE", bufs=4) as pE,
            tc.tile_pool(name="pS", bufs=8) as pS,
            tc.tile_pool(name="pHat", bufs=4) as pHat,
            tc.tile_pool(name="pT", bufs=3) as pT,
            tc.tile_pool(name="pOnce", bufs=1) as pOnce,
        ):
            accs = pOnce.tile([128, 36], f32)

            for b in range(BPC):
                # --- per-batch setup: targets, mask ---
                # NOTE: several TPB structs (TensorScalarPtr/STT) encode only
                # ONE sync-wait command, so each op below is arranged to need
                # at most one cross-engine semaphore wait.
                t_raw = pT.tile([128, PX * NJ], f32, tag="t_raw")  # (px, j)
                m_raw = pT.tile([128, PX], u8, tag="m_raw")
                mf100 = pT.tile([128, PX], f32, tag="mf100")
                tclp = pT.tile([128, NJ, PX], f32, tag="tclp")  # clipped t, j-major
                t2 = pT.tile([128, NJ, PX], f32, tag="t2")  # t'' j-major

                nc.sync.dma_start(t_raw[:, :], tg_v[b])
                nc.sync.dma_start(m_raw[:, :], mk_v[b])

                # prep engine: DVE for batch 0 (shortest pipeline ramp),
                # GpSimd afterwards (keeps DVE free for the custom ops)
                prep = nc.vector if b == 0 else nc.gpsimd
                # mf100 = 100 * mask (waits only on the mask DMA)
                prep.tensor_scalar(
                    out=mf100[:, :],
                    in0=m_raw[:, :],
                    scalar1=100.0,
                    scalar2=None,
                    op0=Alu.mult,
                )
                # tc = min(t, 6.9999), j-major (waits only on t DMA)
                t_raw_v = t_raw[:, :].rearrange("p (px j) -> p j px", j=NJ)
                prep.tensor_scalar(
                    out=tclp[:, :, :],
                    in0=t_raw_v,
                    scalar1=float(BINS - 1) - 1e-4,
                    scalar2=None,
                    op0=Alu.min,
                )
                # t'' = tc + 100*mask (same-engine deps only)
                prep.tensor_tensor(
                    out=t2[:, :, :],
                    in0=tclp[:, :, :],
                    in1=mf100[:, :].unsqueeze(1).broadcast_to((128, NJ, PX)),
                    op=Alu.add,
                )
                # npos accum (DVE; same-engine dep on mf100 only)
                np_scr = pT.tile([128, PX], f32, tag="np_scr")
                nc.vector.tensor_scalar(
                    out=np_scr[:, :],
                    in0=mf100[:, :],
                    scalar1=0.01,
                    scalar2=0.0,
                    op0=Alu.mult,
                    op1=Alu.add,  # reduce op for accum_out
                    accum_out=accs[:, 32 + b : 33 + b],
                )

                for j in range(NJ):
                    u = b * NJ + j
                    L = pL.tile([128, BINS, PX], f32, tag="L")
                    nc.sync.dma_start(L[:, :, :], x_v[b, :, 8 * j : 8 * j + 8, :])

                    # interp: acc[:, u] = sum_k relu(1-|t-k|) * L_k
                    hat_scr = pHat.tile([128, BINS, PX], bf16, tag="hat")
                    nc.vector._custom_dve(
                        hat_op,
                        out=hat_scr[:, :, :],
                        in0=t2[:, j, :].unsqueeze(1).broadcast_to((128, BINS, PX)),
                        in1=L[:, :, :],
                        s0=100.0,
                        s1=1.0,
                        accum_out=accs[:, u : u + 1],
                    )

                    # lse: exp -> pairwise tree -> ln -> masked accumulate
                    E = pE.tile([128, BINS, PX], bf16, tag="E")
                    nc.scalar.activation(E[:, :, :], L[:, :, :], Exp)
                    s16 = pS.tile([128, 4, PX], bf16, tag="s16")
                    s16_eng = nc.gpsimd if (u % 2 == 0) else nc.vector
                    s16_eng.tensor_tensor(
                        out=s16[:, :, :],
                        in0=E[:, 0::2, :],
                        in1=E[:, 1::2, :],
                        op=Alu.add,
                    )
                    s8 = pS.tile([128, 2, PX], bf16, tag="s8")
                    s8_eng = nc.gpsimd if (u % 4 == 1) else nc.vector
                    s8_eng.tensor_tensor(
                        out=s8[:, :, :],
                        in0=s16[:, 0::2, :],
                        in1=s16[:, 1::2, :],
                        op=Alu.add,
                    )
                    s4 = pS.tile([128, PX], f32, tag="s4")
                    nc.vector.tensor_tensor(
                        out=s4[:, :],
                        in0=s8[:, 0, :],
                        in1=s8[:, 1, :],
                        op=Alu.add,
                    )
                    lse = pS.tile([128, PX], f32, tag="lse")
                    nc.scalar.activation(lse[:, :], s4[:, :], Ln)
                    # acc[:, 16+u] = sum(lse * mf100 * 0.01) = sum(lse * mask)
                    lse_scr = pS.tile([128, PX], f32, tag="lse_scr")
                    nc.vector._custom_dve(
                        ttr_op,
                        out=lse_scr[:, :],
                        in0=lse[:, :],
                        in1=mf100[:, :],
                        s0=0.0,
                        s1=0.01,
                        accum_out=accs[:, 16 + u : 17 + u],
                    )

            nc.sync.dma_start(acc_out[:, :], accs[:, :])

    nc.finalize()
    return nc


def _get_nc():
    with _lock:
        if "nc" not in _cache:
            _cache["nc"] = _build_nc()
        return _cache["nc"]


def kernel(reg_logits: np.ndarray, targets: np.ndarray, pos_mask: np.ndarray) -> np.ndarray:
    from concourse.bass_utils import run_bass_kernel_spmd

    nc = _get_nc()

    reg_logits = np.ascontiguousarray(reg_logits, dtype=np.float32).reshape(B, C, HW)
    targets = np.ascontiguousarray(targets, dtype=np.float32)
    mask_u8 = np.ascontiguousarray(pos_mask).astype(np.uint8)

    in_maps = []
    for c in range(NCORES):
        b0 = c * BPC
        in_maps.append(
            {
                "x": reg_logits[b0 : b0 + BPC],
                "tg": targets[b0 : b0 + BPC],
                "mk": mask_u8[b0 : b0 + BPC],
            }
        )

    res = run_bass_kernel_spmd(nc, in_maps, core_ids=list(range(NCORES)))

    tot_interp = 0.0
    tot_lse = 0.0
    npos100 = 0.0
    for r in res.results:
        a = r["acc"].astype(np.float64)
        tot_interp += a[:, :16].sum()
        tot_lse += a[:, 16:32].sum()
        npos100 += a[:, 32:36].sum()

    npos = npos100  # npos accum already scaled to counts
    total = tot_lse - tot_interp
    loss = total / (max(npos, 1.0) * 4.0) if npos > 0 else 0.0
    return np.float32(loss)


if __name__ == "__main__":
    rng = np.random.default_rng(0)
    rl = rng.standard_normal((B, C, H, W), dtype=np.float32)
    tg = (rng.random((B, HW, NJ), dtype=np.float32) * (BINS - 1)).astype(np.float32)
    pm = rng.integers(0, 2, size=(B, HW)).astype(bool)
    print(kernel(reg_logits=rl, targets=tg, pos_mask=pm))
